# revision 1
# baseline (speedup 1.0000x reference)
"""Polynomial flow regularizer loss on 8 Trainium2 NeuronCores.

reference semantics: fit a quadratic polynomial surface (basis
[1, x, y, x^2, x*y, y^2] over a [-1,1]^2 grid) to each (b, c) image of
flow_field (64, 2, 512, 512) via least squares, and return
mean_b(sum_c(mean_pixels((f - fit)^2))).

Math used here: with Phi the (N, 6) basis, G = Phi^T Phi and r = Phi^T f,
the residual energy is  ||f - Phi G^-1 r||^2 = ||f||^2 - r^T G^-1 r.
The basis is separable in (x, y), so r is recoverable from the 3x512
matrix V[a, w] = sum_h y_h^a f[h, w]  (a = 0, 1, 2) via
r_{(a,b)} = sum_w V[a, w] x_w^b.

Device work per image (512x512), with partition p holding rows 4p..4p+4:
  - V via 4 accumulating bf16 TensorE matmuls: lhsT = y-basis slice
    (128, 3), rhs = image sub-row slice (128, 512).
  - sum of squares: ScalarE (Square + accum_out) takes 3 of the 4
    sub-rows, VectorE (mul + reduce) the other, so the elementwise pass
    splits across both engines. Only the total matters, so any
    partition/column decomposition of the partial sums is fine.
Host work: the 6-vector r per image, the 6x6 solve, and the final mean —
a few thousand flops on ~100KB of device output.

Sharding: data-parallel over batch. Core k takes batches [8k, 8k+8)
= 16 images; the host pre-casts to bf16 so each core streams 8.4MB.
Input streams as 2-4MB DMAs on the otherwise-empty sync HWDGE ring
(~400GB/s); outputs leave via scalar/sync/gpsimd rings in parallel at
the tail. Loss contributions are summed on host.
"""

import sys

import numpy as np

sys.path.insert(0, "/opt/trn_rl_repo")

import concourse.bacc as bacc
import concourse.bass as bass
import concourse.tile as tile
from concourse import mybir
from concourse.bass_utils import run_bass_kernel_spmd

B, C, H, W = 64, 2, 512, 512
N_CORES = 8
IMGS = (B // N_CORES) * C  # images per core
HCHUNKS = H // 128  # sub-rows per partition
F32 = mybir.dt.float32

_NC = None


def _build():
    BF16 = mybir.dt.bfloat16
    nc = bacc.Bacc()
    # the host pre-casts the input to bf16: halves the HBM stream, and the
    # loss tolerates it (squares err ~5e-6, fit term is 2e-5 of the loss)
    flow = nc.declare_dram_parameter("flow", [IMGS, H, W], BF16, isOutput=False)
    # [:, :12] interleaved order (h = 4p+s), [:, 12:] chunk order (h = 128t+p)
    ybas = nc.declare_dram_parameter(
        "ybasis", [128, 6 * HCHUNKS], BF16, isOutput=False
    )
    v_out = nc.declare_dram_parameter("v_out", [3, IMGS, W], F32, isOutput=True)
    sq_out = nc.declare_dram_parameter("sq_out", [128, 2 * IMGS], F32, isOutput=True)

    with tile.TileContext(nc) as tc:
        with (
            tc.tile_pool(name="const", bufs=1) as cpool,
            tc.tile_pool(name="img", bufs=3) as ipool,
            tc.tile_pool(name="imglast", bufs=1) as ilpool,
            tc.tile_pool(name="scr", bufs=2) as spool,
            tc.tile_pool(name="psum", bufs=4, space="PSUM") as ppool,
        ):
            yb = cpool.tile([128, 6 * HCHUNKS], BF16)
            nc.scalar.dma_start(out=yb[:], in_=ybas[:])
            # only the TOTAL sum of squares matters for the loss, so the
            # square work can be split arbitrarily across engines/columns
            sq_a = cpool.tile([128, IMGS], F32)
            sq_d = cpool.tile([128, IMGS], F32)
            v_stage = cpool.tile([3, IMGS, W], F32)
            # pair-merged squares leave gap columns; zero them once
            nc.gpsimd.memset(sq_a[:], 0.0)
            nc.gpsimd.memset(sq_d[:], 0.0)

            # 2MB input DMAs stream at ~400GB/s (1MB measured ~334); the
            # last 2MB goes as 1MB + 4x256KB to shorten the compute tail.
            chunks = [4, 4, 4, 2, 1]
            i0 = 0
            for chunk in chunks:
                img = ipool.tile([128, chunk, HCHUNKS, W], BF16, tag="img")
                nc.sync.dma_start(
                    out=img[:],
                    in_=flow[i0 : i0 + chunk].rearrange(
                        "i (p s) w -> p i s w", p=128
                    ),
                )

                # images are processed in pairs sharing one 2-bank PSUM
                # tile so PSUM->SBUF copies (and their sems) halve
                pairs = [
                    list(range(j, min(j + 2, chunk))) for j in range(0, chunk, 2)
                ]
                for pair in pairs:
                    n = len(pair)
                    lo = i0 + pair[0]
                    psum = ppool.tile([3, 2, W], F32)
                    for j in pair:
                        for s in range(HCHUNKS):
                            nc.tensor.matmul(
                                psum[:, j % 2, :],
                                yb[:, 3 * s : 3 * s + 3],
                                img[:, j, s, :],
                                start=(s == 0),
                                stop=(s == HCHUNKS - 1),
                            )

                    # squares over the whole pair: ScalarE takes sub-rows
                    # [0:hi), VectorE [hi:4) (TENSOR_TENSOR_REDUCE crashes
                    # TRN2 here, so plain mul+reduce on DVE instead).
                    # hi alternates 3/2 so the engines carry 5/3 of the 8
                    # sub-rows on average — balanced against the bf16 rates.
                    hi = 3 if (lo // 2) % 2 == 0 else 2
                    j0 = pair[0]
                    scr_a = spool.tile([128, 2, 3, W], BF16, tag="scra")
                    nc.scalar.activation(
                        out=scr_a[:, :n, :hi],
                        in_=img[:, j0 : j0 + n, 0:hi, :],
                        func=mybir.ActivationFunctionType.Square,
                        accum_out=sq_a[:, lo : lo + 1],
                    )
                    scr_d = spool.tile([128, 2, 2, W], BF16, tag="scrd")
                    nc.vector.tensor_mul(
                        scr_d[:, :n, : 4 - hi],
                        img[:, j0 : j0 + n, hi:4, :],
                        img[:, j0 : j0 + n, hi:4, :],
                    )
                    nc.vector.reduce_sum(
                        out=sq_d[:, lo : lo + 1],
                        in_=scr_d[:, :n, : 4 - hi],
                        axis=mybir.AxisListType.XYZ,
                    )
                    nc.vector.tensor_copy(
                        out=v_stage[:, lo : lo + n, :], in_=psum[:, :n, :]
                    )
                    if lo + n == 12:
                        # flush the finished part of V mid-stream; only the
                        # last 4 images' V rides the kernel tail
                        nc.scalar.dma_start(
                            out=v_out[:, 0:12, :], in_=v_stage[:, 0:12, :]
                        )
                i0 += chunk

            # last image: 4 x 256KB row-block DMAs in chunk order
            # (h = 128t+p, contiguous per partition) so each matmul /
            # square starts as soon as its block lands
            i = IMGS - 1
            img = ilpool.tile([128, HCHUNKS, W], BF16)
            for t in range(HCHUNKS):
                nc.sync.dma_start(
                    out=img[:, t, :],
                    in_=flow[i, 128 * t : 128 * (t + 1), :],
                )
            psum = ppool.tile([3, W], F32)
            for t in range(HCHUNKS):
                nc.tensor.matmul(
                    psum[:, :],
                    yb[:, 3 * (HCHUNKS + t) : 3 * (HCHUNKS + t) + 3],
                    img[:, t, :],
                    start=(t == 0),
                    stop=(t == HCHUNKS - 1),
                )
            scr_a = spool.tile([128, 3, W], BF16, tag="scra")
            nc.scalar.activation(
                out=scr_a[:],
                in_=img[:, 0:3, :],
                func=mybir.ActivationFunctionType.Square,
                accum_out=sq_a[:, i : i + 1],
            )
            scr_d = spool.tile([128, W], BF16, tag="scrd")
            nc.vector.tensor_mul(
                scr_d[:], img[:, 3, :], img[:, 3, :]
            )
            nc.vector.reduce_sum(
                out=sq_d[:, i : i + 1], in_=scr_d[:], axis=mybir.AxisListType.X
            )
            nc.vector.tensor_copy(out=v_stage[:, i, :], in_=psum[:])

            # three engines issue the outputs in parallel so the tail pays
            # one issue+receipt latency, not three back-to-back
            nc.scalar.dma_start(out=v_out[:, 12:, :], in_=v_stage[:, 12:, :])
            nc.sync.dma_start(out=sq_out[:, 0:IMGS], in_=sq_a[:])
            nc.gpsimd.dma_start(out=sq_out[:, IMGS:], in_=sq_d[:])
    nc.finalize()
    return nc
    return nc


def _ybasis():
    import ml_dtypes

    y = np.linspace(-1.0, 1.0, H, dtype=np.float32)
    Y = np.empty((128, 6 * HCHUNKS), dtype=np.float32)
    for s in range(HCHUNKS):
        seg = y[s::HCHUNKS]  # interleaved: h = HCHUNKS*p + s
        Y[:, 3 * s + 0] = 1.0
        Y[:, 3 * s + 1] = seg
        Y[:, 3 * s + 2] = seg * seg
    for t in range(HCHUNKS):
        seg = y[128 * t : 128 * (t + 1)]  # chunked: h = 128*t + p
        Y[:, 3 * (HCHUNKS + t) + 0] = 1.0
        Y[:, 3 * (HCHUNKS + t) + 1] = seg
        Y[:, 3 * (HCHUNKS + t) + 2] = seg * seg
    return Y.astype(ml_dtypes.bfloat16)


def _gram():
    # G = Phi^T Phi for basis [1, x, y, x^2, x*y, y^2]; exploits
    # separability: each entry is (sum_h phi_y) * (sum_w phi_x). The y-side
    # uses the bf16-rounded basis the device actually applies.
    import ml_dtypes

    y = np.linspace(-1.0, 1.0, H, dtype=np.float32)
    yb = y.astype(ml_dtypes.bfloat16).astype(np.float64)
    y2b = (y * y).astype(ml_dtypes.bfloat16).astype(np.float64)
    yv = [np.ones_like(yb), yb, y2b]
    x = np.linspace(-1.0, 1.0, W, dtype=np.float32).astype(np.float64)
    xv = [np.ones_like(x), x, x * x]
    # exponents (ay, ax) per basis fn
    e = [(0, 0), (0, 1), (1, 0), (0, 2), (1, 1), (2, 0)]
    G = np.empty((6, 6))
    for j in range(6):
        for k in range(6):
            G[j, k] = (yv[e[j][0]] * yv[e[k][0]]).sum() * (
                xv[e[j][1]] * xv[e[k][1]]
            ).sum()
    return G


def _run(shards, ybasis=None, trace=False, **kwargs):
    """shards: (8, IMGS, H, W) float32. Returns BassKernelResults."""
    global _NC
    if _NC is None:
        _NC = _build()
    if ybasis is None:
        ybasis = _ybasis()
    in_maps = [
        {"flow": np.ascontiguousarray(shards[k]), "ybasis": ybasis}
        for k in range(N_CORES)
    ]
    return run_bass_kernel_spmd(_NC, in_maps, list(range(N_CORES)), trace=trace, **kwargs)


def kernel(flow_field: np.ndarray) -> np.ndarray:
    import ml_dtypes

    global _NC
    flow = np.asarray(flow_field, dtype=np.float32)
    assert flow.shape == (B, C, H, W)
    shards = np.ascontiguousarray(
        flow.reshape(N_CORES, IMGS, H, W).astype(ml_dtypes.bfloat16)
    )

    # rare transient NRT device errors recover on a clean retry
    last_err = None
    for attempt in range(3):
        try:
            res = _run(shards)
            break
        except Exception as e:  # noqa: BLE001
            last_err = e
            _NC = None
    else:
        raise last_err

    G = _gram()
    x = np.linspace(-1.0, 1.0, W, dtype=np.float32).astype(np.float64)
    Xb = np.stack([np.ones_like(x), x, x * x], axis=1)  # (W, 3)

    Ginv = np.linalg.inv(G)
    total = 0.0
    for k in range(N_CORES):
        v = np.asarray(res.results[k]["v_out"], dtype=np.float64)  # (3, IMGS, W)
        sq = np.asarray(res.results[k]["sq_out"], dtype=np.float64)  # (128, IMGS)
        M = np.einsum("aiw,wb->iab", v, Xb)  # (IMGS, 3, 3)
        r = np.stack(
            [M[:, 0, 0], M[:, 0, 1], M[:, 1, 0], M[:, 0, 2], M[:, 1, 1], M[:, 2, 0]],
            axis=1,
        )  # (IMGS, 6)
        fit_energy = np.einsum("ij,jk,ik->i", r, Ginv, r)  # r^T G^-1 r
        total += float(sq.sum() - fit_energy.sum())

    loss = total / (H * W) / B
    return np.asarray(loss, dtype=np.float32)



# revision 4
# speedup vs baseline: 1.0784x; 1.0784x over previous
"""Polynomial flow regularizer loss on 8 Trainium2 NeuronCores — fp8 version.

reference semantics: fit a quadratic polynomial surface (basis
[1, x, y, x^2, x*y, y^2] over a [-1,1]^2 grid) to each (b, c) image of
flow_field (64, 2, 512, 512) via least squares, and return
mean_b(sum_c(mean_pixels((f - fit)^2))).

Math: with Phi the (N, 6) basis, G = Phi^T Phi and r = Phi^T f, the
residual energy is ||f||^2 - r^T G^-1 r.  The basis separates in (x, y),
so r is recoverable on host from V[a, w] = sum_h y_h^a f[h, w] (a=0,1,2).

fp8 design (vs the bf16 baseline):
  - host pre-casts to fp8 e4m3 (TRN flavor, max 240): 4.19MB per core,
    half the bf16 HBM stream.  Loss bias from input quantization is
    ~-8e-4 relative (tolerance 2e-2).
  - V via DoubleRow fp8 matmuls: contraction pairs two 128-row chunks
    per instruction, one matmul per (row-parity, group).  Row layout
    h = 256*cp + 2p + t so each partition's DMA lines are 1KB contiguous.
  - ||f||^2 split across three engines by w-columns, each ONE pass with a
    hardware accumulator: ACT activation(Square, accum_out), DVE and Pool
    scalar_tensor_tensor(x*1*x, accum_out).  Rates 0.833/1.042/1.39
    ns/elem -> split 214/170/128 of 512 columns.
  - V leaves PSUM via a small bf16 staging copy split across the three
    engines by the same w-ranges, then a sync-queue DMA to DRAM.
Host work: r assembly from V, the 6x6 solve, final reduction.
"""

import sys

import numpy as np

sys.path.insert(0, "/opt/trn_rl_repo")

import concourse.bacc as bacc
import concourse.bass as bass
import concourse.tile as tile
from concourse import mybir
from concourse.bass_utils import run_bass_kernel_spmd

B, C, H, W = 64, 2, 512, 512
N_CORES = 8
IMGS = (B // N_CORES) * C  # images per core = 16
F32 = mybir.dt.float32
FP8 = mybir.dt.float8e4
BF16 = mybir.dt.bfloat16

GROUPS = [4, 4, 4, 2, 1, 1]
NG = len(GROUPS)
# w-column split of the squares across ACT / DVE (rates 0.833 / 1.042
# ns/elem).  Pool supports neither scalar_tensor_tensor nor free-axis
# reduce on TRN2; for now it runs a shadow mul (timing probe only).
# ACT and DVE also carry the V psum->SBUF copy (Pool cannot read PSUM).
WA, WD = 284, 228
WP = 160  # shadow pool columns [W-WP:W), result unused
VCA = 284  # V-copy columns on ACT; DVE takes the rest (228)

_NC = None


def _build():
    nc = bacc.Bacc()
    flow = nc.declare_dram_parameter("flow", [IMGS, H, W], FP8, isOutput=False)
    # ybasis[k, t, cp, m] = (y at row h=256*cp+2k+t) ** m, fp8-rounded
    # basis columns padded 3 -> 16: DoubleRow LDWEIGHTS requires >=16
    # weight columns per half; rows 3..15 of PSUM are garbage, never read
    ybas = nc.declare_dram_parameter("ybasis", [128, 2, 2, 16], FP8, isOutput=False)
    v_out = nc.declare_dram_parameter("v_out", [3, IMGS, W], BF16, isOutput=True)
    sq_out = nc.declare_dram_parameter("sq_out", [128, 2, NG], F32, isOutput=True)

    with tile.TileContext(nc) as tc:
        with (
            tc.tile_pool(name="const", bufs=1) as cpool,
            tc.tile_pool(name="img", bufs=3) as ipool,
            tc.tile_pool(name="scr", bufs=2) as spool,
            tc.tile_pool(name="psum", bufs=2, space="PSUM") as ppool,
        ):
            yb = cpool.tile([128, 2, 2, 16], FP8)
            nc.scalar.dma_start(out=yb[:], in_=ybas[:])
            sq_a = cpool.tile([128, NG], F32)
            sq_d = cpool.tile([128, NG], F32)

            g0 = 0
            for g, n in enumerate(GROUPS):
                # img[p, i, cp, t, w]: row h = 256*cp + 2p + t of image i.
                # Per (p, i, cp) the (t, w) block is 1KB contiguous on both
                # sides, and (i, cp) merge on the DRAM side, so the DMA is a
                # 3-dim full-rate pattern.
                img = ipool.tile([128, n, 2, 2, W], FP8, tag="img")
                nc.sync.dma_start(
                    out=img[:],
                    in_=flow[g0 : g0 + n].rearrange(
                        "i (cp p t) w -> p i cp (t w)", cp=2, p=128, t=2
                    ),
                )

                # V: per image one DoubleRow matmul per row parity t, each
                # contracting both cp chunks at once.  A matmul output must
                # stay inside one PSUM bank, hence per-image outputs.
                # Weights only change per parity: 2 LDWEIGHTS per group.
                psum = ppool.tile([16, 4, W], F32, tag="v")
                for t in range(2):
                    for j in range(n):
                        nc.tensor.matmul(
                            psum[:, j, :],
                            yb[:, t, :, :],
                            img[:, j, :, t, :],
                            start=(t == 0),
                            stop=(t == 1),
                            perf_mode=mybir.MatmulPerfMode.DoubleRow,
                        )

                # squares: one pass per engine, hardware accumulators
                scr_a = spool.tile([128, 4, 2, 2, WA], BF16, tag="scra")
                nc.scalar.activation(
                    out=scr_a[:, :n, :, :, :],
                    in_=img[:, :, :, :, 0:WA],
                    func=mybir.ActivationFunctionType.Square,
                    accum_out=sq_a[:, g : g + 1],
                )
                scr_d = spool.tile([128, 4, 2, 2, WD], BF16, tag="scrd")
                nc.vector.scalar_tensor_tensor(
                    out=scr_d[:, :n, :, :, :],
                    in0=img[:, :, :, :, WA : WA + WD],
                    scalar=1.0,
                    in1=img[:, :, :, :, WA : WA + WD],
                    op0=mybir.AluOpType.mult,
                    op1=mybir.AluOpType.mult,
                    accum_out=sq_d[:, g : g + 1],
                )
                # shadow probe: Pool mul at v2's intended share, fp8 out
                scr_p = spool.tile([128, 2, 4, 2, WP], FP8, tag="scrp")
                nc.gpsimd.tensor_tensor(
                    out=scr_p[:, :, :n, :, :].rearrange("p cp i t w -> p i cp t w"),
                    in0=img[:, :, :, :, W - WP : W],
                    in1=img[:, :, :, :, W - WP : W],
                    op=mybir.AluOpType.mult,
                )

                # V: PSUM -> bf16 staging, copy split across the three
                # engines by w-range (DMA cannot read PSUM), then one DMA
                # on the otherwise-idle sync queue.
                v_sb = spool.tile([3, 4, W], BF16, tag="vsb")
                nc.scalar.copy(out=v_sb[:, :n, 0:VCA], in_=psum[0:3, :n, 0:VCA])
                nc.vector.tensor_copy(
                    out=v_sb[:, :n, VCA:W], in_=psum[0:3, :n, VCA:W]
                )
                nc.sync.dma_start(
                    out=v_out[:, g0 : g0 + n, :], in_=v_sb[:, :n, :]
                )
                g0 += n

            nc.scalar.dma_start(out=sq_out[:, 0, :], in_=sq_a[:])
            nc.sync.dma_start(out=sq_out[:, 1, :], in_=sq_d[:])
    nc.finalize()
    return nc


def _yvals():
    """(q(y), q(y*y)) on the fp8 e4m3 grid, f64."""
    import ml_dtypes

    y = np.linspace(-1.0, 1.0, H, dtype=np.float32)
    qy = y.astype(ml_dtypes.float8_e4m3).astype(np.float64)
    qy2 = (y * y).astype(ml_dtypes.float8_e4m3).astype(np.float64)
    return qy, qy2


def _ybasis():
    import ml_dtypes

    qy, qy2 = _yvals()
    Y = np.zeros((128, 2, 2, 16), dtype=np.float64)
    for t in range(2):
        for cp in range(2):
            h = 256 * cp + 2 * np.arange(128) + t
            Y[:, t, cp, 0] = 1.0
            Y[:, t, cp, 1] = qy[h]
            Y[:, t, cp, 2] = qy2[h]
    return Y.astype(ml_dtypes.float8_e4m3)


def _gram():
    # G = Phi^T Phi for the basis the device actually applies: y-side on the
    # fp8 grid, x-side exact f64.  Each entry factorizes into y-sum * x-sum.
    qy, qy2 = _yvals()
    yv = [np.ones_like(qy), qy, qy2]
    x = np.linspace(-1.0, 1.0, W, dtype=np.float32).astype(np.float64)
    xv = [np.ones_like(x), x, x * x]
    e = [(0, 0), (0, 1), (1, 0), (0, 2), (1, 1), (2, 0)]
    G = np.empty((6, 6))
    for j in range(6):
        for k in range(6):
            G[j, k] = (yv[e[j][0]] * yv[e[k][0]]).sum() * (
                xv[e[j][1]] * xv[e[k][1]]
            ).sum()
    return G


def _run(shards, ybasis=None, trace=False, **kwargs):
    """shards: (8, IMGS, H, W) float32-or-fp8. Returns BassKernelResults."""
    import ml_dtypes

    global _NC
    if _NC is None:
        _NC = _build()
    if ybasis is None:
        ybasis = _ybasis()
    shards = np.asarray(shards)
    if shards.dtype != ml_dtypes.float8_e4m3:
        shards = shards.astype(ml_dtypes.float8_e4m3)
    in_maps = [
        {"flow": np.ascontiguousarray(shards[k]), "ybasis": ybasis}
        for k in range(N_CORES)
    ]
    return run_bass_kernel_spmd(_NC, in_maps, list(range(N_CORES)), trace=trace, **kwargs)


def kernel(flow_field: np.ndarray) -> np.ndarray:
    import ml_dtypes

    global _NC
    flow = np.asarray(flow_field, dtype=np.float32)
    assert flow.shape == (B, C, H, W)
    shards = np.ascontiguousarray(
        flow.reshape(N_CORES, IMGS, H, W).astype(ml_dtypes.float8_e4m3)
    )

    # rare transient NRT device errors recover on a clean retry
    last_err = None
    for attempt in range(3):
        try:
            res = _run(shards)
            break
        except Exception as e:  # noqa: BLE001
            last_err = e
            _NC = None
    else:
        raise last_err

    G = _gram()
    x = np.linspace(-1.0, 1.0, W, dtype=np.float32).astype(np.float64)
    Xb = np.stack([np.ones_like(x), x, x * x], axis=1)  # (W, 3)

    Ginv = np.linalg.inv(G)
    total = 0.0
    for k in range(N_CORES):
        v = np.asarray(res.results[k]["v_out"], dtype=np.float64)  # (3, IMGS, W)
        sq = np.asarray(res.results[k]["sq_out"], dtype=np.float64)  # (128, 3, NG)
        M = np.einsum("aiw,wb->iab", v, Xb)  # (IMGS, 3, 3)
        r = np.stack(
            [M[:, 0, 0], M[:, 0, 1], M[:, 1, 0], M[:, 0, 2], M[:, 1, 1], M[:, 2, 0]],
            axis=1,
        )  # (IMGS, 6)
        fit_energy = np.einsum("ij,jk,ik->i", r, Ginv, r)  # r^T G^-1 r
        total += float(sq.sum() - fit_energy.sum())

    loss = total / (H * W) / B
    return np.asarray(loss, dtype=np.float32)


# revision 5
# speedup vs baseline: 1.3795x; 1.2792x over previous
"""Polynomial flow regularizer loss on 8 Trainium2 NeuronCores — fp8 version.

reference semantics: fit a quadratic polynomial surface (basis
[1, x, y, x^2, x*y, y^2] over a [-1,1]^2 grid) to each (b, c) image of
flow_field (64, 2, 512, 512) via least squares, and return
mean_b(sum_c(mean_pixels((f - fit)^2))).

Math: with Phi the (N, 6) basis, G = Phi^T Phi and r = Phi^T f, the
residual energy is ||f||^2 - r^T G^-1 r.  The basis separates in (x, y),
so r is recoverable on host from V[a, w] = sum_h y_h^a f[h, w] (a=0,1,2).

fp8 design (vs the bf16 baseline):
  - host pre-casts to fp8 e4m3 (TRN flavor, max 240): 4.19MB per core,
    half the bf16 HBM stream.  Loss bias from input quantization is
    ~-8e-4 relative (tolerance 2e-2).
  - V via DoubleRow fp8 matmuls: contraction pairs two 128-row chunks
    per instruction, one matmul per (row-parity, group).  Row layout
    h = 256*cp + 2p + t so each partition's DMA lines are 1KB contiguous.
  - ||f||^2 split across three engines by w-columns, each ONE pass with a
    hardware accumulator: ACT activation(Square, accum_out), DVE and Pool
    scalar_tensor_tensor(x*1*x, accum_out).  Rates 0.833/1.042/1.39
    ns/elem -> split 214/170/128 of 512 columns.
  - V leaves PSUM via a small bf16 staging copy split across the three
    engines by the same w-ranges, then a sync-queue DMA to DRAM.
Host work: r assembly from V, the 6x6 solve, final reduction.
"""

import sys

import numpy as np

sys.path.insert(0, "/opt/trn_rl_repo")

import concourse.bacc as bacc
import concourse.bass as bass
import concourse.tile as tile
from concourse import mybir
from concourse.bass_utils import run_bass_kernel_spmd

B, C, H, W = 64, 2, 512, 512
N_CORES = 8
IMGS = (B // N_CORES) * C  # images per core = 16
F32 = mybir.dt.float32
FP8 = mybir.dt.float8e4
BF16 = mybir.dt.bfloat16

GROUPS = [4, 4, 4, 2, 1, 1]
NG = len(GROUPS)
# w-column split of the squares across ACT / DVE (rates 0.833 / 1.042
# ns/elem).  Pool supports neither scalar_tensor_tensor nor free-axis
# reduce on TRN2; for now it runs a shadow mul (timing probe only).
# ACT and DVE also carry the V psum->SBUF copy (Pool cannot read PSUM).
WA, WD = 267, 245
VCA = 284  # V-copy columns on ACT; DVE takes the rest (228)

_NC = None


def _build():
    nc = bacc.Bacc()
    flow = nc.declare_dram_parameter("flow", [IMGS, H, W], FP8, isOutput=False)
    # ybasis[k, t, cp, m] = (y at row h=256*cp+2k+t) ** m, fp8-rounded
    # basis columns padded 3 -> 16: DoubleRow LDWEIGHTS requires >=16
    # weight columns per half; rows 3..15 of PSUM are garbage, never read
    ybas = nc.declare_dram_parameter("ybasis", [128, 2, 2, 16], FP8, isOutput=False)
    v_out = nc.declare_dram_parameter("v_out", [3, IMGS, W], BF16, isOutput=True)
    sq_out = nc.declare_dram_parameter("sq_out", [128, 2, NG], F32, isOutput=True)

    with tile.TileContext(nc) as tc:
        with (
            tc.tile_pool(name="const", bufs=1) as cpool,
            tc.tile_pool(name="img", bufs=3) as ipool,
            tc.tile_pool(name="scr", bufs=2) as spool,
            tc.tile_pool(name="psum", bufs=2, space="PSUM") as ppool,
        ):
            yb = cpool.tile([128, 2, 2, 16], FP8)
            nc.scalar.dma_start(out=yb[:], in_=ybas[:])
            sq_a = cpool.tile([128, NG], F32)
            sq_d = cpool.tile([128, NG], F32)

            g0 = 0
            for g, n in enumerate(GROUPS):
                # img[p, i, cp, t, w]: row h = 256*cp + 2p + t of image i.
                # Per (p, i, cp) the (t, w) block is 1KB contiguous on both
                # sides, and (i, cp) merge on the DRAM side, so the DMA is a
                # 3-dim full-rate pattern.
                img = ipool.tile([128, n, 2, 2, W], FP8, tag="img")
                # alternate input DMAs between the sync HWDGE ring and the
                # gpsimd SWDGE ring so the two streams run in parallel
                dma_eng = nc.sync if g % 2 == 0 else nc.gpsimd
                dma_eng.dma_start(
                    out=img[:],
                    in_=flow[g0 : g0 + n].rearrange(
                        "i (cp p t) w -> p i cp (t w)", cp=2, p=128, t=2
                    ),
                )

                # V: per image one DoubleRow matmul per row parity t, each
                # contracting both cp chunks at once.  A matmul output must
                # stay inside one PSUM bank, hence per-image outputs.
                # Weights only change per parity: 2 LDWEIGHTS per group.
                psum = ppool.tile([16, 4, W], F32, tag="v")
                for t in range(2):
                    for j in range(n):
                        nc.tensor.matmul(
                            psum[:, j, :],
                            yb[:, t, :, :],
                            img[:, j, :, t, :],
                            start=(t == 0),
                            stop=(t == 1),
                            perf_mode=mybir.MatmulPerfMode.DoubleRow,
                        )

                # squares: one pass per engine, hardware accumulators
                scr_a = spool.tile([128, 4, 2, 2, WA], BF16, tag="scra")
                nc.scalar.activation(
                    out=scr_a[:, :n, :, :, :],
                    in_=img[:, :, :, :, 0:WA],
                    func=mybir.ActivationFunctionType.Square,
                    accum_out=sq_a[:, g : g + 1],
                )
                scr_d = spool.tile([128, 4, 2, 2, WD], BF16, tag="scrd")
                nc.vector.scalar_tensor_tensor(
                    out=scr_d[:, :n, :, :, :],
                    in0=img[:, :, :, :, WA : WA + WD],
                    scalar=1.0,
                    in1=img[:, :, :, :, WA : WA + WD],
                    op0=mybir.AluOpType.mult,
                    op1=mybir.AluOpType.mult,
                    accum_out=sq_d[:, g : g + 1],
                )
                # V: PSUM -> bf16 staging, copy split across the three
                # engines by w-range (DMA cannot read PSUM), then one DMA
                # on the otherwise-idle sync queue.
                v_sb = spool.tile([3, 4, W], BF16, tag="vsb")
                nc.scalar.copy(out=v_sb[:, :n, 0:VCA], in_=psum[0:3, :n, 0:VCA])
                nc.vector.tensor_copy(
                    out=v_sb[:, :n, VCA:W], in_=psum[0:3, :n, VCA:W]
                )
                nc.sync.dma_start(
                    out=v_out[:, g0 : g0 + n, :], in_=v_sb[:, :n, :]
                )
                g0 += n

            nc.scalar.dma_start(out=sq_out[:, 0, :], in_=sq_a[:])
            nc.sync.dma_start(out=sq_out[:, 1, :], in_=sq_d[:])
    nc.finalize()
    return nc


def _yvals():
    """(q(y), q(y*y)) on the fp8 e4m3 grid, f64."""
    import ml_dtypes

    y = np.linspace(-1.0, 1.0, H, dtype=np.float32)
    qy = y.astype(ml_dtypes.float8_e4m3).astype(np.float64)
    qy2 = (y * y).astype(ml_dtypes.float8_e4m3).astype(np.float64)
    return qy, qy2


def _ybasis():
    import ml_dtypes

    qy, qy2 = _yvals()
    Y = np.zeros((128, 2, 2, 16), dtype=np.float64)
    for t in range(2):
        for cp in range(2):
            h = 256 * cp + 2 * np.arange(128) + t
            Y[:, t, cp, 0] = 1.0
            Y[:, t, cp, 1] = qy[h]
            Y[:, t, cp, 2] = qy2[h]
    return Y.astype(ml_dtypes.float8_e4m3)


def _gram():
    # G = Phi^T Phi for the basis the device actually applies: y-side on the
    # fp8 grid, x-side exact f64.  Each entry factorizes into y-sum * x-sum.
    qy, qy2 = _yvals()
    yv = [np.ones_like(qy), qy, qy2]
    x = np.linspace(-1.0, 1.0, W, dtype=np.float32).astype(np.float64)
    xv = [np.ones_like(x), x, x * x]
    e = [(0, 0), (0, 1), (1, 0), (0, 2), (1, 1), (2, 0)]
    G = np.empty((6, 6))
    for j in range(6):
        for k in range(6):
            G[j, k] = (yv[e[j][0]] * yv[e[k][0]]).sum() * (
                xv[e[j][1]] * xv[e[k][1]]
            ).sum()
    return G


def _run(shards, ybasis=None, trace=False, **kwargs):
    """shards: (8, IMGS, H, W) float32-or-fp8. Returns BassKernelResults."""
    import ml_dtypes

    global _NC
    if _NC is None:
        _NC = _build()
    if ybasis is None:
        ybasis = _ybasis()
    shards = np.asarray(shards)
    if shards.dtype != ml_dtypes.float8_e4m3:
        shards = shards.astype(ml_dtypes.float8_e4m3)
    in_maps = [
        {"flow": np.ascontiguousarray(shards[k]), "ybasis": ybasis}
        for k in range(N_CORES)
    ]
    return run_bass_kernel_spmd(_NC, in_maps, list(range(N_CORES)), trace=trace, **kwargs)


def kernel(flow_field: np.ndarray) -> np.ndarray:
    import ml_dtypes

    global _NC
    flow = np.asarray(flow_field, dtype=np.float32)
    assert flow.shape == (B, C, H, W)
    shards = np.ascontiguousarray(
        flow.reshape(N_CORES, IMGS, H, W).astype(ml_dtypes.float8_e4m3)
    )

    # rare transient NRT device errors recover on a clean retry
    last_err = None
    for attempt in range(3):
        try:
            res = _run(shards)
            break
        except Exception as e:  # noqa: BLE001
            last_err = e
            _NC = None
    else:
        raise last_err

    G = _gram()
    x = np.linspace(-1.0, 1.0, W, dtype=np.float32).astype(np.float64)
    Xb = np.stack([np.ones_like(x), x, x * x], axis=1)  # (W, 3)

    Ginv = np.linalg.inv(G)
    total = 0.0
    for k in range(N_CORES):
        v = np.asarray(res.results[k]["v_out"], dtype=np.float64)  # (3, IMGS, W)
        sq = np.asarray(res.results[k]["sq_out"], dtype=np.float64)  # (128, 3, NG)
        M = np.einsum("aiw,wb->iab", v, Xb)  # (IMGS, 3, 3)
        r = np.stack(
            [M[:, 0, 0], M[:, 0, 1], M[:, 1, 0], M[:, 0, 2], M[:, 1, 1], M[:, 2, 0]],
            axis=1,
        )  # (IMGS, 6)
        fit_energy = np.einsum("ij,jk,ik->i", r, Ginv, r)  # r^T G^-1 r
        total += float(sq.sum() - fit_energy.sum())

    loss = total / (H * W) / B
    return np.asarray(loss, dtype=np.float32)


# revision 7
# speedup vs baseline: 1.4992x; 1.0868x over previous
"""Polynomial flow regularizer loss on 8 Trainium2 NeuronCores — fp8 version.

reference semantics: fit a quadratic polynomial surface (basis
[1, x, y, x^2, x*y, y^2] over a [-1,1]^2 grid) to each (b, c) image of
flow_field (64, 2, 512, 512) via least squares, and return
mean_b(sum_c(mean_pixels((f - fit)^2))).

Math: with Phi the (N, 6) basis, G = Phi^T Phi and r = Phi^T f, the
residual energy is ||f||^2 - r^T G^-1 r.  The basis separates in (x, y),
so r is recoverable on host from V[a, w] = sum_h y_h^a f[h, w] (a=0,1,2).

fp8 design (vs the bf16 baseline).  The chip power-throttles when all
engines run hot (≈50% util cap for half the kernel), so total
engine-seconds is the currency:
  - host pre-casts to fp8 e4m3 (TRN flavor): 4.19MB per core, half the
    bf16 HBM stream.  Loss bias ~-8e-4 relative (tolerance 2e-2).
  - V via DoubleRow fp8 matmuls (2 elems/lane/cycle): contraction pairs
    the two 256-row halves; one matmul per (image, row-parity).  Weight
    tile zero-padded 3->16 columns (DoubleRow LDWEIGHTS ISA minimum).
    Row layout h = 256*cp + 2p + t keeps DMA lines 1KB contiguous.
  - V outputs stripe-packed into ONE psum bank per group by ROTATING the
    basis inside the weight tile: image j's basis sits at weight columns
    3j..3j+2 (zeros elsewhere), every matmul accumulates into the same
    (16, 512) block at base partition 0 (the ISA rejects DoubleRow dst
    offsets), so the mandatory PSUM->SBUF copy is one op over 512 free
    elems per group, engine-alternated, instead of n*512.
  - ||f||^2: one pass per engine with hardware accumulators: ACT
    activation(Square, accum_out) on w[0:281], DVE scalar_tensor_tensor
    (x*1*x, accum_out) on w[281:512].  Pool's mul measured 3.2 ns/elem
    (vs ACT 0.90 / DVE 1.08) — worse than useless under the power cap.
  - input stream alternates between the sync HWDGE ring and the gpsimd
    SWDGE ring (~220 GB/s each); first group is small so compute starts
    early.
Host work: r assembly from V, the 6x6 solve, final reduction.
"""

import sys

import numpy as np

sys.path.insert(0, "/opt/trn_rl_repo")

import concourse.bacc as bacc
import concourse.bass as bass
import concourse.tile as tile
from concourse import mybir
from concourse.bass_utils import run_bass_kernel_spmd

B, C, H, W = 64, 2, 512, 512
N_CORES = 8
IMGS = (B // N_CORES) * C  # images per core = 16
F32 = mybir.dt.float32
FP8 = mybir.dt.float8e4
BF16 = mybir.dt.bfloat16

GROUPS = [2, 3, 3, 3, 3, 1, 1]
NG = len(GROUPS)
# w-column split of the squares across ACT / DVE (measured 0.90 / 1.08
# ns/elem incl. overheads); both engines also alternate the V copy.
WA, WD = 276, 236

_NC = None


def _pn(n):
    """partitions used by n stripes: image j at partitions 3j..3j+2."""
    return 3 * n


def _build():
    nc = bacc.Bacc()
    flow = nc.declare_dram_parameter("flow", [IMGS, H, W], FP8, isOutput=False)
    # ybasis[k, t, cp, m] = (y at row h=256*cp+2k+t) ** m, fp8-rounded,
    # ybasis[k, j, t, cp, m]: weight set for group-image j, parity t:
    # basis value at column m=3j+a, zeros elsewhere (m padded to 16,
    # the DoubleRow LDWEIGHTS ISA minimum)
    ybas = nc.declare_dram_parameter(
        "ybasis", [128, 3, 2, 2, 16], FP8, isOutput=False
    )
    # v_out[3j+a, g, w] = V[a, image g0+j, w]
    v_out = nc.declare_dram_parameter("v_out", [16, NG, W], BF16, isOutput=True)
    sq_out = nc.declare_dram_parameter("sq_out", [128, 2, NG], F32, isOutput=True)

    with tile.TileContext(nc) as tc:
        with (
            tc.tile_pool(name="const", bufs=1) as cpool,
            tc.tile_pool(name="img", bufs=3) as ipool,
            tc.tile_pool(name="scr", bufs=2) as spool,
            tc.tile_pool(name="psum", bufs=2, space="PSUM") as ppool,
        ):
            yb = cpool.tile([128, 3, 2, 2, 16], FP8)
            nc.scalar.dma_start(out=yb[:], in_=ybas[:])
            sq_a = cpool.tile([128, NG], F32)
            sq_d = cpool.tile([128, NG], F32)

            g0 = 0
            for g, n in enumerate(GROUPS):
                # img[p, i, cp, t, w]: row h = 256*cp + 2p + t of image i.
                # (t, w) is 1KB contiguous on both sides; (i, cp) merge on
                # the DRAM side -> 3-dim full-rate DMA pattern.
                img = ipool.tile([128, n, 2, 2, W], FP8, tag="img")
                dma_eng = nc.sync if g % 2 == 0 else nc.gpsimd
                dma_eng.dma_start(
                    out=img[:],
                    in_=flow[g0 : g0 + n].rearrange(
                        "i (cp p t) w -> p i cp (t w)", cp=2, p=128, t=2
                    ),
                )

                # V: per (image, parity) one DoubleRow matmul contracting
                # both cp halves; image j's weight set routes its basis to
                # output rows 3j..3j+2, zeros elsewhere, so all 2n matmuls
                # accumulate into one shared (16, W) block.
                psum = ppool.tile([16, W], F32, tag="v")
                for j in range(n):
                    for t in range(2):
                        nc.tensor.matmul(
                            psum[:],
                            yb[:, j, t, :, :],
                            img[:, j, :, t, :],
                            start=(j == 0 and t == 0),
                            stop=(j == n - 1 and t == 1),
                            perf_mode=mybir.MatmulPerfMode.DoubleRow,
                        )

                # squares: one pass per engine, hardware accumulators
                scr_a = spool.tile([128, 3, 2, 2, WA], BF16, tag="scra")
                nc.scalar.activation(
                    out=scr_a[:, :n, :, :, :],
                    in_=img[:, :, :, :, 0:WA],
                    func=mybir.ActivationFunctionType.Square,
                    accum_out=sq_a[:, g : g + 1],
                )
                scr_d = spool.tile([128, 3, 2, 2, WD], BF16, tag="scrd")
                nc.vector.scalar_tensor_tensor(
                    out=scr_d[:, :n, :, :, :],
                    in0=img[:, :, :, :, WA:W],
                    scalar=1.0,
                    in1=img[:, :, :, :, WA:W],
                    op0=mybir.AluOpType.mult,
                    op1=mybir.AluOpType.mult,
                    accum_out=sq_d[:, g : g + 1],
                )

                # V exit: one bf16 staging copy over the whole stripe block
                # (free size 512 regardless of n), engines alternating, then
                # one DMA on the sync queue.
                pn = _pn(n)
                v_sb = spool.tile([16, W], BF16, tag="vsb")
                if g % 2 == 0:
                    nc.scalar.copy(out=v_sb[0:pn, :], in_=psum[0:pn, :])
                else:
                    nc.vector.tensor_copy(out=v_sb[0:pn, :], in_=psum[0:pn, :])
                nc.sync.dma_start(out=v_out[0:pn, g, :], in_=v_sb[0:pn, :])
                g0 += n

            nc.scalar.dma_start(out=sq_out[:, 0, :], in_=sq_a[:])
            nc.sync.dma_start(out=sq_out[:, 1, :], in_=sq_d[:])
    nc.finalize()
    return nc


def _yvals():
    """(q(y), q(y*y)) on the fp8 e4m3 grid, f64."""
    import ml_dtypes

    y = np.linspace(-1.0, 1.0, H, dtype=np.float32)
    qy = y.astype(ml_dtypes.float8_e4m3).astype(np.float64)
    qy2 = (y * y).astype(ml_dtypes.float8_e4m3).astype(np.float64)
    return qy, qy2


def _ybasis():
    import ml_dtypes

    qy, qy2 = _yvals()
    Y = np.zeros((128, 3, 2, 2, 16), dtype=np.float64)
    for j in range(3):
        for t in range(2):
            for cp in range(2):
                h = 256 * cp + 2 * np.arange(128) + t
                Y[:, j, t, cp, 3 * j + 0] = 1.0
                Y[:, j, t, cp, 3 * j + 1] = qy[h]
                Y[:, j, t, cp, 3 * j + 2] = qy2[h]
    return Y.astype(ml_dtypes.float8_e4m3)


def _gram():
    # G = Phi^T Phi for the basis the device actually applies: y-side on the
    # fp8 grid, x-side exact f64.  Each entry factorizes into y-sum * x-sum.
    qy, qy2 = _yvals()
    yv = [np.ones_like(qy), qy, qy2]
    x = np.linspace(-1.0, 1.0, W, dtype=np.float32).astype(np.float64)
    xv = [np.ones_like(x), x, x * x]
    e = [(0, 0), (0, 1), (1, 0), (0, 2), (1, 1), (2, 0)]
    G = np.empty((6, 6))
    for j in range(6):
        for k in range(6):
            G[j, k] = (yv[e[j][0]] * yv[e[k][0]]).sum() * (
                xv[e[j][1]] * xv[e[k][1]]
            ).sum()
    return G


def _extract_v(v_raw):
    """v_raw: (16, NG, W) -> V (3, IMGS, W) f64."""
    V = np.empty((3, IMGS, W), dtype=np.float64)
    g0 = 0
    for g, n in enumerate(GROUPS):
        for j in range(n):
            for a in range(3):
                V[a, g0 + j, :] = v_raw[3 * j + a, g, :]
        g0 += n
    return V


def _run(shards, ybasis=None, trace=False, **kwargs):
    """shards: (8, IMGS, H, W) float32-or-fp8. Returns BassKernelResults."""
    import ml_dtypes

    global _NC
    if _NC is None:
        _NC = _build()
    if ybasis is None:
        ybasis = _ybasis()
    shards = np.asarray(shards)
    if shards.dtype != ml_dtypes.float8_e4m3:
        shards = shards.astype(ml_dtypes.float8_e4m3)
    in_maps = [
        {"flow": np.ascontiguousarray(shards[k]), "ybasis": ybasis}
        for k in range(N_CORES)
    ]
    return run_bass_kernel_spmd(_NC, in_maps, list(range(N_CORES)), trace=trace, **kwargs)


def kernel(flow_field: np.ndarray) -> np.ndarray:
    import ml_dtypes

    global _NC
    flow = np.asarray(flow_field, dtype=np.float32)
    assert flow.shape == (B, C, H, W)
    shards = np.ascontiguousarray(
        flow.reshape(N_CORES, IMGS, H, W).astype(ml_dtypes.float8_e4m3)
    )

    # rare transient NRT device errors recover on a clean retry
    last_err = None
    for attempt in range(3):
        try:
            res = _run(shards)
            break
        except Exception as e:  # noqa: BLE001
            last_err = e
            _NC = None
    else:
        raise last_err

    G = _gram()
    x = np.linspace(-1.0, 1.0, W, dtype=np.float32).astype(np.float64)
    Xb = np.stack([np.ones_like(x), x, x * x], axis=1)  # (W, 3)

    Ginv = np.linalg.inv(G)
    total = 0.0
    for k in range(N_CORES):
        v = _extract_v(np.asarray(res.results[k]["v_out"], dtype=np.float64))
        sq = np.asarray(res.results[k]["sq_out"], dtype=np.float64)  # (128, 2, NG)
        M = np.einsum("aiw,wb->iab", v, Xb)  # (IMGS, 3, 3)
        r = np.stack(
            [M[:, 0, 0], M[:, 0, 1], M[:, 1, 0], M[:, 0, 2], M[:, 1, 1], M[:, 2, 0]],
            axis=1,
        )  # (IMGS, 6)
        fit_energy = np.einsum("ij,jk,ik->i", r, Ginv, r)  # r^T G^-1 r
        total += float(sq.sum() - fit_energy.sum())

    loss = total / (H * W) / B
    return np.asarray(loss, dtype=np.float32)


# revision 8
# speedup vs baseline: 1.5156x; 1.0109x over previous
"""Polynomial flow regularizer loss on 8 Trainium2 NeuronCores — fp8 version.

reference semantics: fit a quadratic polynomial surface (basis
[1, x, y, x^2, x*y, y^2] over a [-1,1]^2 grid) to each (b, c) image of
flow_field (64, 2, 512, 512) via least squares, and return
mean_b(sum_c(mean_pixels((f - fit)^2))).

Math: with Phi the (N, 6) basis, G = Phi^T Phi and r = Phi^T f, the
residual energy is ||f||^2 - r^T G^-1 r.  The basis separates in (x, y),
so r is recoverable on host from V[a, w] = sum_h y_h^a f[h, w] (a=0,1,2).

fp8 design (vs the bf16 baseline).  The chip power-throttles when all
engines run hot (≈50% util cap for half the kernel), so total
engine-seconds is the currency:
  - host pre-casts to fp8 e4m3 (TRN flavor): 4.19MB per core, half the
    bf16 HBM stream.  Loss bias ~-8e-4 relative (tolerance 2e-2).
  - V via DoubleRow fp8 matmuls (2 elems/lane/cycle): contraction pairs
    the two 256-row halves; one matmul per (image, row-parity).  Weight
    tile zero-padded 3->16 columns (DoubleRow LDWEIGHTS ISA minimum).
    Row layout h = 256*cp + 2p + t keeps DMA lines 1KB contiguous.
  - V outputs stripe-packed into ONE psum bank per group by ROTATING the
    basis inside the weight tile: image j's basis sits at weight columns
    3j..3j+2 (zeros elsewhere), every matmul accumulates into the same
    (16, 512) block at base partition 0 (the ISA rejects DoubleRow dst
    offsets), so the mandatory PSUM->SBUF copy is one op over 512 free
    elems per group, engine-alternated, instead of n*512.
  - ||f||^2: one pass per engine with hardware accumulators: ACT
    activation(Square, accum_out) on w[0:281], DVE scalar_tensor_tensor
    (x*1*x, accum_out) on w[281:512].  Pool's mul measured 3.2 ns/elem
    (vs ACT 0.90 / DVE 1.08) — worse than useless under the power cap.
  - input stream alternates between the sync HWDGE ring and the gpsimd
    SWDGE ring (~220 GB/s each); first group is small so compute starts
    early.
Host work: r assembly from V, the 6x6 solve, final reduction.
"""

import sys

import numpy as np

sys.path.insert(0, "/opt/trn_rl_repo")

import concourse.bacc as bacc
import concourse.bass as bass
import concourse.tile as tile
from concourse import mybir
from concourse.bass_utils import run_bass_kernel_spmd

B, C, H, W = 64, 2, 512, 512
N_CORES = 8
IMGS = (B // N_CORES) * C  # images per core = 16
F32 = mybir.dt.float32
FP8 = mybir.dt.float8e4
BF16 = mybir.dt.bfloat16

GROUPS = [1, 2, 3, 3, 3, 3, 1]
NG = len(GROUPS)
# w-column split of the squares across ACT / DVE (measured 0.90 / 1.08
# ns/elem incl. overheads); both engines also alternate the V copy.
WA, WD = 248, 264

_NC = None


def _pn(n):
    """partitions used by n stripes: image j at partitions 3j..3j+2."""
    return 3 * n


def _build():
    nc = bacc.Bacc()
    flow = nc.declare_dram_parameter("flow", [IMGS, H, W], FP8, isOutput=False)
    # ybasis[k, t, cp, m] = (y at row h=256*cp+2k+t) ** m, fp8-rounded,
    # ybasis[k, j, t, cp, m]: weight set for group-image j, parity t:
    # basis value at column m=3j+a, zeros elsewhere (m padded to 16,
    # the DoubleRow LDWEIGHTS ISA minimum)
    ybas = nc.declare_dram_parameter(
        "ybasis", [128, 3, 2, 2, 16], FP8, isOutput=False
    )
    # v_out[3j+a, g, w] = V[a, image g0+j, w]
    v_out = nc.declare_dram_parameter("v_out", [16, NG, W], BF16, isOutput=True)
    sq_out = nc.declare_dram_parameter("sq_out", [128, 2, NG], F32, isOutput=True)

    with tile.TileContext(nc) as tc:
        with (
            tc.tile_pool(name="const", bufs=1) as cpool,
            tc.tile_pool(name="img", bufs=4) as ipool,
            tc.tile_pool(name="scr", bufs=2) as spool,
            tc.tile_pool(name="psum", bufs=4, space="PSUM") as ppool,
        ):
            yb = cpool.tile([128, 3, 2, 2, 16], FP8)
            nc.scalar.dma_start(out=yb[:], in_=ybas[:])
            sq_a = cpool.tile([128, NG], F32)
            sq_d = cpool.tile([128, NG], F32)
            # all groups' V stripes stage here; ONE output DMA at the end
            # (per-group DMAs on the sync queue block later input DMAs)
            v_all = cpool.tile([16, NG, W], BF16)

            g0 = 0
            for g, n in enumerate(GROUPS):
                # img[p, i, cp, t, w]: row h = 256*cp + 2p + t of image i.
                # (t, w) is 1KB contiguous on both sides; (i, cp) merge on
                # the DRAM side -> 3-dim full-rate DMA pattern.
                img = ipool.tile([128, n, 2, 2, W], FP8, tag="img")
                dma_eng = nc.sync if g % 2 == 0 else nc.gpsimd
                dma_eng.dma_start(
                    out=img[:],
                    in_=flow[g0 : g0 + n].rearrange(
                        "i (cp p t) w -> p i cp (t w)", cp=2, p=128, t=2
                    ),
                )

                # V: per (image, parity) one DoubleRow matmul contracting
                # both cp halves; image j's weight set routes its basis to
                # output rows 3j..3j+2, zeros elsewhere, so all 2n matmuls
                # accumulate into one shared (16, W) block.
                psum = ppool.tile([16, W], F32, tag="v")
                for j in range(n):
                    for t in range(2):
                        nc.tensor.matmul(
                            psum[:],
                            yb[:, j, t, :, :],
                            img[:, j, :, t, :],
                            start=(j == 0 and t == 0),
                            stop=(j == n - 1 and t == 1),
                            perf_mode=mybir.MatmulPerfMode.DoubleRow,
                        )

                # squares: one pass per engine, hardware accumulators
                scr_a = spool.tile([128, 3, 2, 2, WA], BF16, tag="scra")
                nc.scalar.activation(
                    out=scr_a[:, :n, :, :, :],
                    in_=img[:, :, :, :, 0:WA],
                    func=mybir.ActivationFunctionType.Square,
                    accum_out=sq_a[:, g : g + 1],
                )
                scr_d = spool.tile([128, 3, 2, 2, WD], BF16, tag="scrd")
                nc.vector.scalar_tensor_tensor(
                    out=scr_d[:, :n, :, :, :],
                    in0=img[:, :, :, :, WA:W],
                    scalar=1.0,
                    in1=img[:, :, :, :, WA:W],
                    op0=mybir.AluOpType.mult,
                    op1=mybir.AluOpType.mult,
                    accum_out=sq_d[:, g : g + 1],
                )

                # V exit: one bf16 staging copy over the whole stripe block
                # (free size 512 regardless of n), engines alternating.
                if g % 2 == 0:
                    nc.scalar.copy(out=v_all[:, g, :], in_=psum[:])
                else:
                    nc.vector.tensor_copy(out=v_all[:, g, :], in_=psum[:])
                g0 += n

            nc.sync.dma_start(out=v_out[:], in_=v_all[:])
            nc.gpsimd.dma_start(out=sq_out[:, 0, :], in_=sq_a[:])
            nc.scalar.dma_start(out=sq_out[:, 1, :], in_=sq_d[:])
    nc.finalize()
    return nc


def _yvals():
    """(q(y), q(y*y)) on the fp8 e4m3 grid, f64."""
    import ml_dtypes

    y = np.linspace(-1.0, 1.0, H, dtype=np.float32)
    qy = y.astype(ml_dtypes.float8_e4m3).astype(np.float64)
    qy2 = (y * y).astype(ml_dtypes.float8_e4m3).astype(np.float64)
    return qy, qy2


def _ybasis():
    import ml_dtypes

    qy, qy2 = _yvals()
    Y = np.zeros((128, 3, 2, 2, 16), dtype=np.float64)
    for j in range(3):
        for t in range(2):
            for cp in range(2):
                h = 256 * cp + 2 * np.arange(128) + t
                Y[:, j, t, cp, 3 * j + 0] = 1.0
                Y[:, j, t, cp, 3 * j + 1] = qy[h]
                Y[:, j, t, cp, 3 * j + 2] = qy2[h]
    return Y.astype(ml_dtypes.float8_e4m3)


def _gram():
    # G = Phi^T Phi for the basis the device actually applies: y-side on the
    # fp8 grid, x-side exact f64.  Each entry factorizes into y-sum * x-sum.
    qy, qy2 = _yvals()
    yv = [np.ones_like(qy), qy, qy2]
    x = np.linspace(-1.0, 1.0, W, dtype=np.float32).astype(np.float64)
    xv = [np.ones_like(x), x, x * x]
    e = [(0, 0), (0, 1), (1, 0), (0, 2), (1, 1), (2, 0)]
    G = np.empty((6, 6))
    for j in range(6):
        for k in range(6):
            G[j, k] = (yv[e[j][0]] * yv[e[k][0]]).sum() * (
                xv[e[j][1]] * xv[e[k][1]]
            ).sum()
    return G


def _extract_v(v_raw):
    """v_raw: (16, NG, W) -> V (3, IMGS, W) f64."""
    V = np.empty((3, IMGS, W), dtype=np.float64)
    g0 = 0
    for g, n in enumerate(GROUPS):
        for j in range(n):
            for a in range(3):
                V[a, g0 + j, :] = v_raw[3 * j + a, g, :]
        g0 += n
    return V


def _run(shards, ybasis=None, trace=False, **kwargs):
    """shards: (8, IMGS, H, W) float32-or-fp8. Returns BassKernelResults."""
    import ml_dtypes

    global _NC
    if _NC is None:
        _NC = _build()
    if ybasis is None:
        ybasis = _ybasis()
    shards = np.asarray(shards)
    if shards.dtype != ml_dtypes.float8_e4m3:
        shards = shards.astype(ml_dtypes.float8_e4m3)
    in_maps = [
        {"flow": np.ascontiguousarray(shards[k]), "ybasis": ybasis}
        for k in range(N_CORES)
    ]
    return run_bass_kernel_spmd(_NC, in_maps, list(range(N_CORES)), trace=trace, **kwargs)


def kernel(flow_field: np.ndarray) -> np.ndarray:
    import ml_dtypes

    global _NC
    flow = np.asarray(flow_field, dtype=np.float32)
    assert flow.shape == (B, C, H, W)
    shards = np.ascontiguousarray(
        flow.reshape(N_CORES, IMGS, H, W).astype(ml_dtypes.float8_e4m3)
    )

    # rare transient NRT device errors recover on a clean retry
    last_err = None
    for attempt in range(3):
        try:
            res = _run(shards)
            break
        except Exception as e:  # noqa: BLE001
            last_err = e
            _NC = None
    else:
        raise last_err

    G = _gram()
    x = np.linspace(-1.0, 1.0, W, dtype=np.float32).astype(np.float64)
    Xb = np.stack([np.ones_like(x), x, x * x], axis=1)  # (W, 3)

    Ginv = np.linalg.inv(G)
    total = 0.0
    for k in range(N_CORES):
        v = _extract_v(np.asarray(res.results[k]["v_out"], dtype=np.float64))
        sq = np.asarray(res.results[k]["sq_out"], dtype=np.float64)  # (128, 2, NG)
        M = np.einsum("aiw,wb->iab", v, Xb)  # (IMGS, 3, 3)
        r = np.stack(
            [M[:, 0, 0], M[:, 0, 1], M[:, 1, 0], M[:, 0, 2], M[:, 1, 1], M[:, 2, 0]],
            axis=1,
        )  # (IMGS, 6)
        fit_energy = np.einsum("ij,jk,ik->i", r, Ginv, r)  # r^T G^-1 r
        total += float(sq.sum() - fit_energy.sum())

    loss = total / (H * W) / B
    return np.asarray(loss, dtype=np.float32)
